# revision 36
# baseline (speedup 1.0000x reference)
"""Multi-head attention (B=2, S=2048, D=1024, H=16) on 8 Trainium2 cores.

Sharding: (batch, head-group-of-4) -> 8 cores, Megatron-style. Core c
handles batch c//4 and heads 4*(c%4)..4*(c%4)+3 (d_local = 256 columns of
Wq/Wk/Wv, 256 rows of Wo). Each core computes a partial [2048, 1024]
output; the host sums the 4 partials per batch (row-parallel Wo).

Key-side truncation: only ceil(max(valid_lens)/128) key tiles are ever
computed; the per-batch mask rides the exp as a per-key-partition bias.

Phase C puts queries on PSUM partitions for the P@V matmul: for each
128-query chunk, ctx[q, d(64)+denom(1)] accumulates over key tiles with
V extended by a ones-column, so the softmax denominator is a 65th output
column instead of 64 wasted "ones" rows (halves the P@V tensor time).
Normalized ctx chunks are PE-transposed back to [d, q] for the output
projection. Scores/exp stay in [key, query] layout ([128,1024] exp tiles
keep ScalarE amortized; exp chain runs at high scheduler priority so the
ScalarE stream never starves).

Precision: fp16 streams/weights, fp32 PSUM accumulate (fp8 was measured
to blow the 2e-2 gate in every placement, so everything stays fp16).

The kernel program is built at call time from the actual valid_lens, so
any input values work; shapes are hardcoded to this problem.
"""
import sys
if "/opt/trn_rl_repo" not in sys.path:
    sys.path.insert(0, "/opt/trn_rl_repo")
import os
import time
import numpy as np

B, SQ, SK, D, H, HD = 2, 2048, 2048, 1024, 16, 64
NEG = -1.0e6
N_CORES = 8
DL = 256          # d_local: 4 heads * 64
KD = D // 128     # contraction tiles over D

_NC_CACHE = {}
last_results = None
last_exec_wall_s = None


def _build(KT, prec=None):
    import concourse.bass as bass  # noqa: F401
    import concourse.tile as tile
    from concourse import bacc, mybir

    f32 = mybir.dt.float32
    f16 = mybir.dt.float16
    LK = KT * 128

    # A1 runs per key tile; xk/xv stream in 256-col chunks (>=512B
    # contiguous DMA descriptors); a trailing 128 merges into the previous
    a_chunks = [(t * 128, 128) for t in range(KT)]
    kv_chunks = [(g * 128, min(256, LK - g * 128)) for g in range(0, KT, 2)]
    if len(kv_chunks) > 1 and kv_chunks[-1][1] == 128:
        c0, _ = kv_chunks[-2]
        kv_chunks = kv_chunks[:-2] + [(c0, 384)]
    # V psum groups: pairs of key tiles share one 2KB psum bank
    v_groups = [list(range(g, min(g + 2, KT))) for g in range(0, KT, 2)]

    nc = bacc.Bacc("TRN2", target_bir_lowering=False, debug=False,
                   num_devices=N_CORES)
    xqT = nc.dram_tensor("xqT", [D, SQ], f16, kind="ExternalInput")
    xkT = nc.dram_tensor("xkT", [D, LK], f16, kind="ExternalInput")
    xvT = nc.dram_tensor("xvT", [D, LK], f16, kind="ExternalInput")
    wq = nc.dram_tensor("wq", [D, DL], f16, kind="ExternalInput")
    wk = nc.dram_tensor("wk", [D, DL], f16, kind="ExternalInput")
    wv = nc.dram_tensor("wv", [D, DL], f16, kind="ExternalInput")
    wo = nc.dram_tensor("wo", [DL, D], f16, kind="ExternalInput")
    mask = nc.dram_tensor("mask", [128, KT], f32, kind="ExternalInput")
    ident = nc.dram_tensor("ident", [128, 128], f16, kind="ExternalInput")
    out = nc.dram_tensor("out", [SQ, D], f16, kind="ExternalOutput")

    with tile.TileContext(nc) as tc:
        with tc.tile_pool(name="singles", bufs=1) as sg:
            wk_sb = sg.tile([128, KD, DL], f16)
            wq_sb = sg.tile([128, KD, DL], f16)
            wv_sb = sg.tile([128, KD, DL], f16)
            wo_sb = sg.tile([128, DL // 128, D], f16)
            mask_sb = sg.tile([128, KT], f32)
            ident_sb = sg.tile([128, 128], f16)
            kt_sb = sg.tile([128, 2, LK], f16)      # K^T  [d_local, key]
            qt_sb = sg.tile([128, 2, SQ], f16)      # Q^T  [d_local, query]
            v_sb = sg.tile([128, KT, 4, 65], f16)   # V    [key, head, d | ones]
            ctxT_sb = sg.tile([128, 2, SQ], f16)    # Ctx^T normalized
            xk_full = sg.tile([128, KD, LK], f16)
            xv_full = sg.tile([128, KD, LK], f16)
            xq_full = sg.tile([128, KD, SQ], f16)

            # DMA issue order = stream consumption order; wk/xk split fine
            # so phase A1 can start ~3us in.
            nc.sync.dma_start(out=wk_sb[:, 0:4, :],
                              in_=wk[0:512, :].rearrange("(k p) j -> p k j", p=128))
            nc.sync.dma_start(out=xk_full[:, :, 0:kv_chunks[0][1]],
                              in_=xkT[:, 0:kv_chunks[0][1]].rearrange("(k p) j -> p k j", p=128))
            nc.sync.dma_start(out=wq_sb, in_=wq[:, :].rearrange("(k p) j -> p k j", p=128))
            for kh in range(2):
                nc.sync.dma_start(
                    out=xq_full[:, kh * 4:(kh + 1) * 4, 0:512],
                    in_=xqT[kh * 512:(kh + 1) * 512, 0:512].rearrange("(k p) j -> p k j", p=128))
            for kh in range(2):
                nc.sync.dma_start(
                    out=xq_full[:, kh * 4:(kh + 1) * 4, 512:1024],
                    in_=xqT[kh * 512:(kh + 1) * 512, 512:1024].rearrange("(k p) j -> p k j", p=128))
            nc.sync.dma_start(out=wk_sb[:, 4:8, :],
                              in_=wk[512:1024, :].rearrange("(k p) j -> p k j", p=128))
            nc.sync.dma_start(out=mask_sb, in_=mask[:, :])
            nc.sync.dma_start(out=ident_sb, in_=ident[:, :])
            for c0, cw in kv_chunks[1:]:
                nc.sync.dma_start(out=xk_full[:, :, c0:c0 + cw],
                                  in_=xkT[:, c0:c0 + cw].rearrange("(k p) j -> p k j", p=128))
            nc.sync.dma_start(out=wv_sb, in_=wv[:, :].rearrange("(k p) j -> p k j", p=128))
            for c0, cw in kv_chunks:
                nc.sync.dma_start(out=xv_full[:, :, c0:c0 + cw],
                                  in_=xvT[:, c0:c0 + cw].rearrange("(k p) j -> p k j", p=128))
            for p in range(2, SQ // 512):
                nc.sync.dma_start(
                    out=xq_full[:, :, p * 512:(p + 1) * 512],
                    in_=xqT[:, p * 512:(p + 1) * 512].rearrange("(k p) j -> p k j", p=128))
            nc.sync.dma_start(out=wo_sb, in_=wo[:, :].rearrange("(k p) j -> p k j", p=128))

            nc.vector.memset(v_sb, 1.0)
            # dummy exp: pulls the activation-table load off the critical path
            warm_sb = sg.tile([1, 1], f32)
            nc.vector.memset(warm_sb, 0.0)
            nc.scalar.activation(warm_sb, warm_sb,
                                 mybir.ActivationFunctionType.Exp)

            # PE p-state warmup: dummy matmuls on a memset tile keep the PE
            # busy through the ramp window while the first DMAs land, so
            # real matmuls all run at full clock.
            wsrc = sg.tile([128, 64], f16)
            nc.vector.memset(wsrc, 0.0)

            # ---- single psum pool: "s" scores (2x2 banks), "c" shared
            # 1-bank slots (A1/Q/V accumulators + ctx), "m" shared 1-bank
            # slots (warmup, transposes, out-proj) -> exactly 8 banks ----
            with tc.tile_pool(name="psP", bufs=2, space="PSUM") as psP, \
                 tc.tile_pool(name="ptp", bufs=min(3 * KT + 1, 16)) as ptp, \
                 tc.tile_pool(name="cnp", bufs=17) as cnp, \
                 tc.tile_pool(name="smp", bufs=4) as smp, \
                 tc.tile_pool(name="obp", bufs=8) as obp:
                psS = psC = psM = psP

                # PE p-state warmup: a run of dummy matmuls on one scratch
                # tile (same-tile WAW keeps them back-to-back) spans the
                # DMA-bound start so real matmuls begin at full clock
                wdst = psM.tile([64, 64], f32, tag="m", name="wdst")
                for i in range(110):
                    nc.tensor.matmul(wdst, wsrc, wsrc[:, 0:64],
                                     start=True, stop=True)

                # ---- Phase A1: K^T = (Wk^T blocks) @ Xk^T, [256, LK] ----
                for ci, (c0, cw) in enumerate(a_chunks):
                    for m in range(2):
                        acc = psM.tile([128, 512], f32, tag="m",
                                       name=f"kacc{m}_{ci}")
                        for k in range(KD):
                            nc.tensor.matmul(acc[:, 0:cw],
                                             wk_sb[:, k, m * 128:(m + 1) * 128],
                                             xk_full[:, k, c0:c0 + cw],
                                             start=(k == 0), stop=(k == KD - 1))
                        nc.scalar.copy(kt_sb[:, m, c0:c0 + cw], acc[:, 0:cw])
                    if ci == 1:
                        # keep the PE clock warm through the wk_b/xq0
                        # DMA-bound hole between the early and late A1 chunks
                        for i in range(60):
                            nc.tensor.matmul(wdst, wsrc, wsrc[:, 0:64],
                                             start=True, stop=True)

                def qpass(p, m):
                    qacc = psC.tile([128, 512], f32, tag="c", name=f"qacc{p}_{m}")
                    for k in range(KD):
                        nc.tensor.matmul(qacc,
                                         wq_sb[:, k, m * 128:(m + 1) * 128],
                                         xq_full[:, k, p * 512:(p + 1) * 512],
                                         start=(k == 0), stop=(k == KD - 1))
                    nc.vector.tensor_copy(qt_sb[:, m, p * 512:(p + 1) * 512], qacc)

                def v_unit(g):
                    # V = Xv^T.T @ Wv [key, d_local]; pairs of key tiles
                    # share one psum bank as a single merged group
                    vacc = psC.tile([128, 2, 256], f32, tag="c",
                                    name=f"vacc{g[0]}")
                    last = (len(g) - 1, KD - 1)
                    for k in range(KD):
                        for gi, t in enumerate(g):
                            nc.tensor.matmul(
                                vacc[:, gi, :],
                                xv_full[:, k, t * 128:(t + 1) * 128],
                                wv_sb[:, k, :],
                                start=(k == 0 and gi == 0),
                                stop=((gi, k) == last),
                                skip_group_check=True)
                    for gi, t in enumerate(g):
                        nc.vector.tensor_copy(
                            v_sb[:, t, :, 0:64],
                            vacc[:, gi, :].rearrange("p (h d) -> p h d", h=4))

                def scores_exp(half, hh):
                    mt, mo = hh // 2, 64 * (hh % 2)
                    h0 = half * 1024
                    pts = []
                    for t in range(KT):
                        with tc.high_priority():
                            s = psS.tile([128, 1024], f32, tag="s",
                                         name=f"s{half}_{hh}_{t}")
                            for cq in range(2):
                                nc.tensor.matmul(
                                    s[:, cq * 512:(cq + 1) * 512],
                                    kt_sb[mo:mo + 64, mt, t * 128:(t + 1) * 128],
                                    qt_sb[mo:mo + 64, mt, h0 + cq * 512:h0 + (cq + 1) * 512],
                                    start=True, stop=True)
                            pt_t = ptp.tile([128, 1024], f16, tag="pt",
                                            name=f"pt{half}_{hh}_{t}")
                            nc.scalar.activation(
                                pt_t, s, mybir.ActivationFunctionType.Exp,
                                bias=mask_sb[:, t:t + 1], scale=0.125)
                        pts.append(pt_t)
                    return pts

                ctxn = {}

                def ctx_unit(half, hh, pts, cgs=(0, 1)):
                    mt, mo = hh // 2, 64 * (hh % 2)
                    for cg in cgs:
                        ctx = psC.tile([128, 4, 65], f32, tag="c",
                                       name=f"ctx{half}_{hh}_{cg}")
                        for ci in range(4):
                            c = cg * 4 + ci
                            for t in range(KT):
                                nc.tensor.matmul(
                                    ctx[:, ci, :],
                                    pts[t][:, c * 128:(c + 1) * 128],
                                    v_sb[:, t, hh, :],
                                    start=(ci == 0 and t == 0),
                                    stop=(ci == 3 and t == KT - 1),
                                    skip_group_check=True)
                        rcb = smp.tile([128, 4], f32, tag="r",
                                       name=f"rcb{half}_{hh}_{cg}")
                        nc.vector.reciprocal(rcb, ctx[:, :, 64])
                        for ci in range(4):
                            c = cg * 4 + ci
                            key = (half, mt, c)
                            if key not in ctxn:
                                ctxn[key] = cnp.tile([128, 128], f16, tag="cn",
                                                     name=f"cn{half}_{mt}_{c}")
                            eng = nc.vector if ci % 2 == 0 else nc.gpsimd
                            eng.tensor_scalar_mul(ctxn[key][:, mo:mo + 64],
                                                  ctx[:, ci, 0:64],
                                                  rcb[:, ci:ci + 1])

                def transposes(half, mt, cs):
                    for c in cs:
                        tp = psM.tile([128, 128], f16, tag="m",
                                      name=f"tp{half}_{mt}_{c}")
                        nc.tensor.transpose(tp, ctxn[(half, mt, c)], ident_sb)
                        eng = nc.gpsimd if c % 2 == 0 else nc.vector
                        eng.tensor_copy(
                            ctxT_sb[:, mt, half * 1024 + c * 128:half * 1024 + (c + 1) * 128],
                            tp)

                dseq = [0]

                def d_unit(half, qcs, tail=False):
                    for qc in qcs:
                        qb = half * 1024 + qc * 128
                        ob = obp.tile([128, D], f16, tag="ob", name=f"ob{half}_{qc}")
                        for n in range(2):
                            if tail:
                                pool, tg = [(psC, "c"), (psM, "m"),
                                            (psS, "s")][dseq[0] % 3]
                                dseq[0] += 1
                            else:
                                pool, tg = psM, "m"
                            o = pool.tile([128, 512], f32, tag=tg,
                                          name=f"o{half}_{qc}_{n}")
                            for kk in range(2):
                                nc.tensor.matmul(o,
                                                 ctxT_sb[:, kk, qb:qb + 128],
                                                 wo_sb[:, kk, n * 512:(n + 1) * 512],
                                                 start=(kk == 0), stop=(kk == 1))
                            # evacuate each psum tile with two engines in
                            # parallel so the slot recycles fast; at the
                            # tail rotate all three engines for throughput
                            ob_n = ob[:, n * 512:(n + 1) * 512]
                            if tail:
                                engs = [nc.scalar, nc.vector, nc.gpsimd]
                                e0 = engs[dseq[0] % 3]
                                e1 = engs[(dseq[0] + 1) % 3]
                            else:
                                e0, e1 = nc.gpsimd, nc.vector
                            for eng, sl in ((e0, slice(0, 256)),
                                            (e1, slice(256, 512))):
                                if eng is nc.scalar:
                                    eng.copy(ob_n[:, sl], o[:, sl])
                                else:
                                    eng.tensor_copy(ob_n[:, sl], o[:, sl])
                        nc.sync.dma_start(out=out[qb:qb + 128, :], in_=ob)

                # ---- emission schedule ----
                qpass(0, 0); qpass(1, 0)
                pts = {}
                pts[(0, 0)] = scores_exp(0, 0)
                qpass(0, 1); qpass(1, 1)
                pts[(0, 1)] = scores_exp(0, 1)
                for g in v_groups:
                    v_unit(g)
                ctx_unit(0, 0, pts[(0, 0)])
                ctx_unit(0, 1, pts[(0, 1)])
                transposes(0, 0, range(8))
                pts[(0, 2)] = scores_exp(0, 2)
                qpass(2, 0); qpass(3, 0)
                ctx_unit(0, 2, pts[(0, 2)])
                pts[(0, 3)] = scores_exp(0, 3)
                qpass(2, 1); qpass(3, 1)
                ctx_unit(0, 3, pts[(0, 3)])
                transposes(0, 1, range(8))

                pts[(1, 0)] = scores_exp(1, 0)
                d_unit(0, [0, 1])
                ctx_unit(1, 0, pts[(1, 0)])
                d_unit(0, [2, 3])
                pts[(1, 1)] = scores_exp(1, 1)
                d_unit(0, [4, 5])
                ctx_unit(1, 1, pts[(1, 1)])
                transposes(1, 0, range(8))
                pts[(1, 2)] = scores_exp(1, 2)
                d_unit(0, [6, 7])
                ctx_unit(1, 2, pts[(1, 2)], tail=True)
                pts[(1, 3)] = scores_exp(1, 3)
                ctx_unit(1, 3, pts[(1, 3)], cgs=(0,))
                transposes(1, 1, range(0, 4))
                ctx_unit(1, 3, pts[(1, 3)], cgs=(1,))
                transposes(1, 1, range(4, 8))
                d_unit(1, list(range(8)), tail=True)

    nc.compile()
    return nc


def kernel(**inputs):
    global last_results, last_exec_wall_s
    from concourse.bass_utils import run_bass_kernel_spmd

    # BASS_TRACE needs the axon NTFF hook; disable tracing when the hook
    # module is unavailable so a stray env var cannot crash the run.
    if os.environ.get("BASS_TRACE"):
        try:
            from antenv import axon_hooks  # noqa: F401
        except Exception:
            os.environ["BASS_NEVER_TRACE"] = "1"

    q = np.asarray(inputs["queries"], dtype=np.float32)
    kx = np.asarray(inputs["keys"], dtype=np.float32)
    vx = np.asarray(inputs["values"], dtype=np.float32)
    vl = np.asarray(inputs["valid_lens"], dtype=np.int64).reshape(B)
    Wq = np.asarray(inputs["Wq"], dtype=np.float32)
    Wk = np.asarray(inputs["Wk"], dtype=np.float32)
    Wv = np.asarray(inputs["Wv"], dtype=np.float32)
    Wo = np.asarray(inputs["Wo"], dtype=np.float32)
    assert q.shape == (B, SQ, D) and kx.shape == (B, SK, D) and vx.shape == (B, SK, D)

    lens = np.clip(vl, 1, SK)
    lmax = int(lens.max())
    KT = (lmax + 127) // 128
    LK = KT * 128

    if KT not in _NC_CACHE:
        _NC_CACHE[KT] = _build(KT)
    nc = _NC_CACHE[KT]

    xdt = np.float16
    eye = np.ascontiguousarray(np.eye(128, dtype=xdt))

    in_maps = []
    for c in range(N_CORES):
        b, hg = c // 4, c % 4
        cols = slice(DL * hg, DL * (hg + 1))
        m = np.where(np.arange(LK) < lens[b], 0.0, NEG).astype(np.float32)
        in_maps.append({
            "xqT": np.ascontiguousarray(q[b].T.astype(xdt)),
            "xkT": np.ascontiguousarray(kx[b, :LK].T.astype(xdt)),
            "xvT": np.ascontiguousarray(vx[b, :LK].T.astype(xdt)),
            "wq": np.ascontiguousarray(Wq[:, cols].astype(xdt)),
            "wk": np.ascontiguousarray(Wk[:, cols].astype(xdt)),
            "wv": np.ascontiguousarray(Wv[:, cols].astype(xdt)),
            "wo": np.ascontiguousarray(Wo[cols, :].astype(xdt)),
            "mask": np.ascontiguousarray(m.reshape(KT, 128).T),
            "ident": eye,
        })

    t0 = time.perf_counter()
    res = run_bass_kernel_spmd(nc, in_maps, core_ids=list(range(N_CORES)))
    last_exec_wall_s = time.perf_counter() - t0
    last_results = res

    outs = [res.results[c]["out"].astype(np.float32) for c in range(N_CORES)]
    full = np.stack([outs[0] + outs[1] + outs[2] + outs[3],
                     outs[4] + outs[5] + outs[6] + outs[7]])
    return full.astype(np.float32)
